# revision 27
# baseline (speedup 1.0000x reference)
"""Trainium2 Bass kernel for nn_DFPT_Node (soft binary decision tree).

Full inputs in, full output out; internally data-parallel over 8 NeuronCores
(batch sharded 65536 -> 8 x 8192). Tree params (c, s, dims, leaf_logits) are
baked into compiled constants on the host:

  gate:  g = sigmoid(-4 (x[:,dims] - c)/|s|) = sigmoid(a*x + b),
         a = -4/|s| as a scaled one-hot matmul (fp16 hi/lo split of x for
         precision, K=128), b = -a*c applied exactly via ACT per-partition bias.
  tree:  levels 0-6 batch-major (batch on partitions), levels 7-9 node-major
         (nodes on partitions, batch on free dim) in block (bit-reversed)
         leaf order; level 9 folded into the output matmul in q-basis:
         out = sum_t q_t @ A_t + p9_t @ B_t, accumulated in PSUM via
         batch-major flipped matmuls (lhsT = F batch tile, rhs = M chunk).

The kernel is ACT(sigmoid)-bound: the schedule keeps the Activation engine
saturated (lag-0 modulo schedule: each chunk's deep products + folds are
emitted right behind its sigmoid), so head latency and post-sigmoid drain
are the only overheads.

Output leaves the device batch-major as outt [128, 8, 8, 16] f32 per core.
"""

import numpy as np

B_FULL = 65536
IN_DIM = 64
N_CLASS = 10
MAX_DEPTH = 10
N_CORES = 8
B_CORE = B_FULL // N_CORES      # 8192
SLAB = 1024                     # batch columns processed per slab
N_SLABS = B_CORE // SLAB        # 8
N_CHUNKS = 8                    # node-major chunks of 128 nodes

F16 = np.float16
F32 = np.float32

_CACHE = {}


def _build_tree_layout():
    """pos[d][i] = reference position within level d of block-order index i."""
    pos = [np.array([0], dtype=np.int64)]
    for _ in range(MAX_DEPTH):
        p = pos[-1]
        pos.append(np.concatenate([2 * p, 2 * p + 1]))
    return pos


def _build_constants(c, s, dims, leaf_logits):
    """W chunks [8,128,128] f16, dropped-lo features, M chunks [8,128,16].

    The bias rides inside W as two fp16 rows (126: fp16(b), 127: fp16
    residual) against constant-1.0 xt2 rows, freeing the ACT engine from a
    per-chunk bias operand. To make room, the two features with the
    smallest max|a| lose their x_lo row (fp16-only x for those features;
    validated ~1e-7 effect on output error for this tree).
    """
    pos = _build_tree_layout()
    chunk_nodes = -np.ones((N_CHUNKS, 128), dtype=np.int64)
    for d in range(7):
        base = (1 << d) - 1
        chunk_nodes[0, base: base + (1 << d)] = base + pos[d]
    chunk_nodes[1, :] = 127 + pos[7]
    lvl8 = 255 + pos[8]
    chunk_nodes[2, :] = lvl8[:128]
    chunk_nodes[3, :] = lvl8[128:]
    lvl9 = 511 + pos[9]
    for t in range(4):
        chunk_nodes[4 + t, :] = lvl9[128 * t: 128 * (t + 1)]

    a64 = -4.0 / np.abs(s.astype(np.float64))
    a16 = a64.astype(F16)
    b64 = -a16.astype(np.float64) * c.astype(np.float64)
    b1 = b64.astype(F16)
    b2 = (b64 - b1.astype(np.float64)).astype(F16)

    dims = dims.astype(np.int64)
    feat_max = np.zeros(IN_DIM)
    np.maximum.at(feat_max, dims, np.abs(a64))
    drop = np.argsort(feat_max)[:2]
    lo_feats = np.array([k for k in range(IN_DIM) if k not in drop])
    lo_row = -np.ones(IN_DIM, dtype=np.int64)
    lo_row[lo_feats] = IN_DIM + np.arange(IN_DIM - 2)

    W = np.zeros((N_CHUNKS, 128, 128), dtype=F16)
    ch_idx, col_idx = np.nonzero(chunk_nodes >= 0)
    g_idx = chunk_nodes[ch_idx, col_idx]
    d_idx = dims[g_idx]
    W[ch_idx, d_idx, col_idx] = a16[g_idx]
    has_lo = lo_row[d_idx] >= 0
    W[ch_idx[has_lo], lo_row[d_idx[has_lo]], col_idx[has_lo]] = (
        a16[g_idx[has_lo]])
    W[ch_idx, 126, col_idx] = b1[g_idx]
    W[ch_idx, 127, col_idx] = b2[g_idx]

    L_my = leaf_logits[pos[MAX_DEPTH]].astype(np.float64)  # [1024, 10]
    A = L_my[:512] - L_my[512:]
    Bm = L_my[512:]
    # M chunk i pairs with F chunk i in fold order
    # [l9a, r9a, l9b, r9b, q0, q1, q2, q3]; p9 block order is
    # [l9a, l9b, r9a, r9b] over Bm quarters, q block order over A quarters.
    Mlist = [Bm[0:128], Bm[256:384], Bm[128:256], Bm[384:512],
             A[0:128], A[128:256], A[256:384], A[384:512]]
    M = np.zeros((N_CHUNKS, 128, 16), dtype=F16)
    for i, m in enumerate(Mlist):
        M[i, :, :N_CLASS] = m.astype(F16)
    return W, lo_feats, M


def _build_program():
    import concourse.bass as bass
    import concourse.tile as tile
    from concourse import bacc, mybir

    f16 = mybir.dt.float16
    f32 = mybir.dt.float32
    SIG = mybir.ActivationFunctionType.Sigmoid

    nc = bacc.Bacc("TRN2", target_bir_lowering=False)
    xt2_d = nc.dram_tensor("xt2", [128, B_CORE], f16, kind="ExternalInput")
    wt_d = nc.dram_tensor("wt", [128, N_CHUNKS, 128], f16, kind="ExternalInput")
    mt_d = nc.dram_tensor("mt", [128, N_CHUNKS, 16], f16, kind="ExternalInput")
    out_d = nc.dram_tensor("outt", [128, N_SLABS, 8, 16], f32,
                           kind="ExternalOutput")

    # flat per-slab gate layout: gt[:, ch*1024 + col], so sigmoid tiles can
    # span chunk boundaries (the bias lives inside the W matmul)
    GFLAT = N_CHUNKS * SLAB

    with tile.TileContext(nc) as tc:
        with (
            tc.tile_pool(name="singles", bufs=1) as singles,
            tc.tile_pool(name="work", bufs=3) as work,
            tc.tile_pool(name="gtpool", bufs=4) as gtpool,
            tc.tile_pool(name="fwork", bufs=2) as fwork,
            tc.tile_pool(name="zpsum", bufs=2, space="PSUM") as zpsum,
            tc.tile_pool(name="opsum", bufs=2, space="PSUM") as opsum,
        ):
            # resident constants; ordered so slab 0's z can start ASAP.
            # GPSIMD's DMA queue issues in 25ns (vs ~550 on SP), so the
            # first-z inputs split across both queues for minimum latency.
            w_sb = singles.tile([128, N_CHUNKS, 128], f16)
            nc.gpsimd.dma_start(out=w_sb[:, 0, :], in_=wt_d[:, 0, :])
            xt2 = singles.tile([128, B_CORE], f16)
            nc.sync.dma_start(out=xt2[:, 0:256], in_=xt2_d[:, 0:256])
            nc.sync.dma_start(out=xt2[:, 256:512], in_=xt2_d[:, 256:512])
            nc.sync.dma_start(out=xt2[:, 512:1024], in_=xt2_d[:, 512:1024])
            nc.sync.dma_start(out=w_sb[:, 1:, :], in_=wt_d[:, 1:, :])
            ones0 = singles.tile([128, 8, 1], f16)
            nc.vector.memset(ones0, 1.0)
            sigwarm = singles.tile([1, 1], f16)
            nc.vector.memset(sigwarm, 0.0)
            nc.scalar.activation(sigwarm, sigwarm, SIG, bias=0.0, scale=1.0)
            m_sb = singles.tile([128, N_CHUNKS, 16], f16)
            nc.sync.dma_start(out=m_sb, in_=mt_d[:, :, :])
            # remaining slabs arrive via the (otherwise idle) GPSIMD DMA
            # queue so they do not queue behind the SP constants
            for i in range(1, N_SLABS):
                sl = bass.ts(i, SLAB)
                nc.gpsimd.dma_start(out=xt2[:, sl], in_=xt2_d[:, sl])

            state = {}

            def emit_zacts(sl, c0, c1, split=False):
                """z matmuls + one sigmoid for flat gate cols [c0, c1)."""
                gt = state[("gt", sl)]
                with tc.high_priority():
                    z = zpsum.tile([128, c1 - c0], f32, tag="z",
                                   name=f"z{sl}_{c0}")
                    for j in range(c0 // 512, c1 // 512):
                        ch, h = divmod(j, 2)
                        nc.tensor.matmul(
                            z[:, j * 512 - c0: j * 512 - c0 + 512],
                            lhsT=w_sb[:, ch, :],
                            rhs=xt2[:, sl * SLAB + h * 512:
                                    sl * SLAB + (h + 1) * 512],
                            start=True,
                            stop=True,
                        )
                    if split:
                        mid = (c0 + c1) // 2
                        nc.scalar.activation(gt[:, c0:mid], z[:, 0:mid - c0],
                                             SIG, bias=0.0, scale=1.0)
                        nc.scalar.activation(gt[:, mid:c1],
                                             z[:, mid - c0:c1 - c0],
                                             SIG, bias=0.0, scale=1.0)
                    else:
                        nc.scalar.activation(gt[:, c0:c1], z, SIG,
                                             bias=0.0, scale=1.0)

            def emit_shallow(sl):
                """gt chunk0 -> batch-major, cascade levels 0-6 -> p7t."""
                gt = state[("gt", sl)]
                gsh = work.tile([128, 8, 128], f16, tag="gsh", name=f"gsh{sl}")
                for g in range(8):
                    nc.sync.dma_start(
                        out=gsh[:, g, :],
                        in_=gt[:, g * 128: (g + 1) * 128],
                        transpose=True,
                    )
                prev = ones0[:, 0:8, :]
                for d in range(7):
                    n = 1 << d
                    cur = work.tile([128, 8, 2 * n], f16, tag=f"pb{d + 1}",
                                    name=f"pb{d + 1}_{sl}")
                    gl = gsh[:, :, n - 1: 2 * n - 1]
                    nc.vector.tensor_mul(cur[:, :, 0:n], prev, gl)
                    nc.vector.tensor_sub(cur[:, :, n: 2 * n], prev,
                                         cur[:, :, 0:n])
                    prev = cur
                p7t = work.tile([128, SLAB], f16, tag="p7t", name=f"p7t{sl}")
                for g in range(8):
                    nc.sync.dma_start(
                        out=p7t[:, g * 128: (g + 1) * 128],
                        in_=prev[:, g, :],
                        transpose=True,
                    )
                state[("p7t", sl)] = p7t

            def prefetch(nxt):
                state[("gt", nxt)] = gtpool.tile([128, GFLAT], f16, tag="gt",
                                                 name=f"gt{nxt}")
                emit_zacts(nxt, 0, SLAB)
                emit_shallow(nxt)

            def emit_slab(sl, last):
                gt = state[("gt", sl)]
                op = opsum.tile([128, 8, 16], f32, tag="op", name=f"op{sl}")
                osb = work.tile([128, 8, 16], f32, tag="osb", name=f"osb{sl}")
                F = {}

                def gch(ch, h0=0, h1=SLAB):
                    return gt[:, ch * SLAB + h0: ch * SLAB + h1]

                def ftile(name):
                    t = fwork.tile([128, SLAB], f16, tag=name,
                                   name=f"{name}_{sl}")
                    F[name] = t
                    return t

                def folds(name, m_idx, start=False, stop=False, bsubs=None):
                    # start=True only on the very first matmul of this op
                    # tile: it marks the whole 2KB PSUM zero-region pending,
                    # so each chain's first touch initializes and later
                    # touches accumulate. A second start would clobber
                    # sibling chains sharing the bank.
                    src = F[name]
                    for i, b in enumerate(range(8) if bsubs is None else bsubs):
                        nc.tensor.matmul(
                            op[:, b, 0:N_CLASS],
                            lhsT=src[:, b * 128: (b + 1) * 128],
                            rhs=m_sb[:, m_idx, 0:N_CLASS],
                            start=start and i == 0,
                            stop=stop,
                            skip_group_check=True,
                        )

                # sigmoid tiles for chunks 1-7: 4x1536 + 1x1024 flat columns
                bounds = [1024, 2560, 4096, 5632, 7168, 8192]
                for u in range(5):
                    emit_zacts(sl, bounds[u], bounds[u + 1],
                               split=(last and u == 4))
                    if u == 0:
                        # prefetch chunk-0 gates + shallow chains two windows
                        # ahead: the gsh/cascade/p7t chain has ~5us of
                        # latency (two DMA-semaphore hops + serial cascade)
                        if sl + 2 < N_SLABS:
                            prefetch(sl + 2)
                        # ch1 complete
                        p7t = state[("p7t", sl)]
                        l8 = ftile("l8")
                        nc.vector.tensor_mul(l8, p7t, gch(1))
                        r8 = ftile("r8")
                        nc.vector.tensor_sub(r8, p7t, l8)
                    elif u == 1:
                        # ch2 + ch3 complete
                        l9a = ftile("l9a")
                        nc.vector.tensor_mul(l9a, F["l8"], gch(2))
                        r9a = ftile("r9a")
                        nc.vector.tensor_sub(r9a, F["l8"], l9a)
                        l9b = ftile("l9b")
                        nc.gpsimd.tensor_mul(l9b, F["r8"], gch(3))
                        # r9b on DVE: it feeds q3, the tail-critical chain
                        r9b = ftile("r9b")
                        nc.vector.tensor_sub(r9b, F["r8"], l9b)
                        folds("l9a", 0, start=True)
                        folds("r9a", 1)
                        folds("l9b", 2)
                        folds("r9b", 3)
                    elif u == 2:
                        # ch4 complete
                        q0 = ftile("q0")
                        nc.gpsimd.tensor_mul(q0, F["l9a"], gch(4))
                        folds("q0", 4)
                    elif u == 3:
                        # ch5 + ch6 complete
                        q1 = ftile("q1")
                        nc.gpsimd.tensor_mul(q1, F["l9b"], gch(5))
                        q2 = ftile("q2")
                        nc.vector.tensor_mul(q2, F["r9a"], gch(6))
                        folds("q1", 5)
                        folds("q2", 6)
                    elif u == 4:
                        # ch7 complete
                        q3 = ftile("q3")
                        if last:
                            for h in range(2):
                                hs = bass.ts(h, 512)
                                nc.vector.tensor_mul(
                                    q3[:, hs], F["r9b"][:, hs],
                                    gch(7, h * 512, (h + 1) * 512))
                                folds("q3", 7, stop=True,
                                      bsubs=range(4 * h, 4 * h + 4))
                                bs = slice(4 * h, 4 * h + 4)
                                nc.vector.tensor_copy(
                                    osb[:, bs, :], op[:, bs, :])
                                nc.gpsimd.dma_start(
                                    out=out_d[:, sl, bs, :],
                                    in_=osb[:, bs, :],
                                )
                        else:
                            nc.vector.tensor_mul(q3, F["r9b"], gch(7))
                            folds("q3", 7, stop=True)
                            nc.vector.tensor_copy(osb, op)
                            nc.gpsimd.dma_start(
                                out=out_d[:, sl, :, :], in_=osb[:, :, :])

            # slab 0 chunk-0 in a 256 + 768 piece pair so the very first
            # sigmoid fires as soon as the first 256 x-columns land
            state[("gt", 0)] = gtpool.tile([128, GFLAT], f16,
                                           tag="gt", name="gt0")
            gt0 = state[("gt", 0)]
            with tc.high_priority():
                za = zpsum.tile([128, 256], f32, tag="z", name="z0_head")
                nc.tensor.matmul(za, lhsT=w_sb[:, 0, :], rhs=xt2[:, 0:256],
                                 start=True, stop=True)
                nc.scalar.activation(gt0[:, 0:256], za, SIG,
                                     bias=0.0, scale=1.0)
                zb = zpsum.tile([128, 768], f32, tag="z", name="z0_head2")
                # one start per PSUM bank; within a started bank the first
                # touch initializes (bytes are pending-zero)
                for i, st in ((0, True), (1, False), (2, True)):
                    nc.tensor.matmul(
                        zb[:, i * 256: (i + 1) * 256],
                        lhsT=w_sb[:, 0, :],
                        rhs=xt2[:, 256 + i * 256: 512 + i * 256],
                        start=st,
                        stop=True,
                    )
                nc.scalar.activation(gt0[:, 256:1024], zb, SIG,
                                     bias=0.0, scale=1.0)
            emit_shallow(0)
            state[("gt", 1)] = gtpool.tile([128, GFLAT], f16,
                                           tag="gt", name="gt1")
            emit_zacts(1, 0, SLAB)
            emit_shallow(1)
            for sl in range(N_SLABS):
                emit_slab(sl, last=(sl == N_SLABS - 1))

    nc.finalize()
    return nc


def _get_program():
    if "nc" not in _CACHE:
        _CACHE["nc"] = _build_program()
    return _CACHE["nc"]


def kernel(x, c, s, leaf_logits, dims, max_depth):
    from concourse.bass_utils import run_bass_kernel_spmd

    assert int(max_depth) == MAX_DEPTH
    x = np.asarray(x, dtype=F32)
    c = np.asarray(c, dtype=F32)
    s = np.asarray(s, dtype=F32)
    leaf_logits = np.asarray(leaf_logits, dtype=F32)
    dims = np.asarray(dims)

    W, lo_feats, M = _build_constants(c, s, dims, leaf_logits)
    wt = np.ascontiguousarray(W.transpose(1, 0, 2))            # [128, 8, 128]
    mt = np.ascontiguousarray(M.transpose(1, 0, 2))            # [128, 8, 16]

    in_maps = []
    for core in range(N_CORES):
        xc = x[core * B_CORE: (core + 1) * B_CORE]             # [8192, 64]
        xT = np.ascontiguousarray(xc.T).astype(F32)            # [64, 8192]
        x_hi = xT.astype(F16)
        x_lo = (xT - x_hi.astype(F32)).astype(F16)
        xt2 = np.empty((128, B_CORE), dtype=F16)               # [128, 8192]
        xt2[:IN_DIM] = x_hi
        xt2[IN_DIM:126] = x_lo[lo_feats]
        xt2[126:] = 1.0                                        # bias rows
        in_maps.append({"xt2": xt2, "wt": wt, "mt": mt})

    _CACHE["in_maps"] = in_maps
    nc = _get_program()
    res = run_bass_kernel_spmd(nc, in_maps, core_ids=list(range(N_CORES)))

    out = np.empty((B_FULL, N_CLASS), dtype=F32)
    for core in range(N_CORES):
        outt = res.results[core]["outt"]                # [128, 8, 8, 16] f32
        arr = outt[:, :, :, :N_CLASS].transpose(1, 2, 0, 3).reshape(
            B_CORE, N_CLASS)
        out[core * B_CORE: (core + 1) * B_CORE] = arr.astype(F32)
    return out


# revision 29
# speedup vs baseline: 1.0057x; 1.0057x over previous
"""Trainium2 Bass kernel for nn_DFPT_Node (soft binary decision tree).

Full inputs in, full output out; internally data-parallel over 8 NeuronCores
(batch sharded 65536 -> 8 x 8192). Tree params (c, s, dims, leaf_logits) are
baked into compiled constants on the host:

  gate:  g = sigmoid(-4 (x[:,dims] - c)/|s|) = sigmoid(a*x + b),
         a = -4/|s| as a scaled one-hot matmul (fp16 hi/lo split of x for
         precision); b rides inside W as two fp16 rows against constant-1
         xt2 rows, so the sigmoid needs no per-chunk ACT bias and one ACT
         op can span chunk boundaries.
  tree:  levels 0-6 batch-major (batch on partitions), levels 7-9 node-major
         (nodes on partitions, batch on free dim) in block (bit-reversed)
         leaf order; level 9 folded into the output matmul in q-basis:
         out = sum_t q_t @ A_t + p9_t @ B_t, accumulated in PSUM via
         batch-major flipped matmuls (lhsT = F batch tile, rhs = M chunk
         [128,10] -> ~free on the PE cost model).

The kernel is ACT(sigmoid)-bound: 8.4M sigmoid elements/core at
0.83ns/col/128 lanes ~= 54.6us processing. The schedule keeps ACT saturated:
z tiles of 1536 columns (3 PSUM banks, double buffered, next to a 2-bank op
accumulator) amortize ACT init; chunk-0 gates + the shallow cascade prefetch
two windows ahead (the gsh/cascade/p7t chain has ~5us of DMA-semaphore
latency); each later chunk's deep products and fold matmuls follow its
sigmoid within the window (lag-0), so only head latency and a ~3.3us
post-sigmoid drain remain.

Output leaves the device batch-major as outt [128, 8, 8, 16] f32 per core.
"""

import numpy as np

B_FULL = 65536
IN_DIM = 64
N_CLASS = 10
MAX_DEPTH = 10
N_CORES = 8
B_CORE = B_FULL // N_CORES      # 8192
SLAB = 1024                     # batch columns processed per slab
N_SLABS = B_CORE // SLAB        # 8
N_CHUNKS = 8                    # node-major chunks of 128 nodes

F16 = np.float16
F32 = np.float32

_CACHE = {}


def _build_tree_layout():
    """pos[d][i] = reference position within level d of block-order index i."""
    pos = [np.array([0], dtype=np.int64)]
    for _ in range(MAX_DEPTH):
        p = pos[-1]
        pos.append(np.concatenate([2 * p, 2 * p + 1]))
    return pos


def _build_constants(c, s, dims, leaf_logits):
    """W chunks [8,128,128] f16, dropped-lo features, M chunks [8,128,16].

    The bias rides inside W as two fp16 rows (126: fp16(b), 127: fp16
    residual) against constant-1.0 xt2 rows, freeing the ACT engine from a
    per-chunk bias operand. To make room, the two features with the
    smallest max|a| lose their x_lo row (fp16-only x for those features;
    validated ~1e-7 effect on output error for this tree).
    """
    pos = _build_tree_layout()
    chunk_nodes = -np.ones((N_CHUNKS, 128), dtype=np.int64)
    for d in range(7):
        base = (1 << d) - 1
        chunk_nodes[0, base: base + (1 << d)] = base + pos[d]
    chunk_nodes[1, :] = 127 + pos[7]
    lvl8 = 255 + pos[8]
    chunk_nodes[2, :] = lvl8[:128]
    chunk_nodes[3, :] = lvl8[128:]
    lvl9 = 511 + pos[9]
    for t in range(4):
        chunk_nodes[4 + t, :] = lvl9[128 * t: 128 * (t + 1)]

    a64 = -4.0 / np.abs(s.astype(np.float64))
    a16 = a64.astype(F16)
    b64 = -a16.astype(np.float64) * c.astype(np.float64)
    b1 = b64.astype(F16)
    b2 = (b64 - b1.astype(np.float64)).astype(F16)

    dims = dims.astype(np.int64)
    feat_max = np.zeros(IN_DIM)
    np.maximum.at(feat_max, dims, np.abs(a64))
    drop = np.argsort(feat_max)[:2]
    lo_feats = np.array([k for k in range(IN_DIM) if k not in drop])
    lo_row = -np.ones(IN_DIM, dtype=np.int64)
    lo_row[lo_feats] = IN_DIM + np.arange(IN_DIM - 2)

    W = np.zeros((N_CHUNKS, 128, 128), dtype=F16)
    ch_idx, col_idx = np.nonzero(chunk_nodes >= 0)
    g_idx = chunk_nodes[ch_idx, col_idx]
    d_idx = dims[g_idx]
    W[ch_idx, d_idx, col_idx] = a16[g_idx]
    has_lo = lo_row[d_idx] >= 0
    W[ch_idx[has_lo], lo_row[d_idx[has_lo]], col_idx[has_lo]] = (
        a16[g_idx[has_lo]])
    W[ch_idx, 126, col_idx] = b1[g_idx]
    W[ch_idx, 127, col_idx] = b2[g_idx]

    L_my = leaf_logits[pos[MAX_DEPTH]].astype(np.float64)  # [1024, 10]
    A = L_my[:512] - L_my[512:]
    Bm = L_my[512:]
    # M chunk i pairs with F chunk i in fold order
    # [l9a, r9a, l9b, r9b, q0, q1, q2, q3]; p9 block order is
    # [l9a, l9b, r9a, r9b] over Bm quarters, q block order over A quarters.
    Mlist = [Bm[0:128], Bm[256:384], Bm[128:256], Bm[384:512],
             A[0:128], A[128:256], A[256:384], A[384:512]]
    M = np.zeros((N_CHUNKS, 128, 16), dtype=F16)
    for i, m in enumerate(Mlist):
        M[i, :, :N_CLASS] = m.astype(F16)
    return W, lo_feats, M


def _build_program():
    import concourse.bass as bass
    import concourse.tile as tile
    from concourse import bacc, mybir

    f16 = mybir.dt.float16
    f32 = mybir.dt.float32
    SIG = mybir.ActivationFunctionType.Sigmoid

    nc = bacc.Bacc("TRN2", target_bir_lowering=False)
    xt2_d = nc.dram_tensor("xt2", [128, B_CORE], f16, kind="ExternalInput")
    wt_d = nc.dram_tensor("wt", [128, N_CHUNKS, 128], f16, kind="ExternalInput")
    mt_d = nc.dram_tensor("mt", [128, N_CHUNKS, 16], f16, kind="ExternalInput")
    out_d = nc.dram_tensor("outt", [128, N_SLABS, 8, 16], f32,
                           kind="ExternalOutput")

    # flat per-slab gate layout: gt[:, ch*1024 + col], so sigmoid tiles can
    # span chunk boundaries (the bias lives inside the W matmul)
    GFLAT = N_CHUNKS * SLAB

    with tile.TileContext(nc) as tc:
        with (
            tc.tile_pool(name="singles", bufs=1) as singles,
            tc.tile_pool(name="work", bufs=3) as work,
            tc.tile_pool(name="gtpool", bufs=4) as gtpool,
            tc.tile_pool(name="fwork", bufs=2) as fwork,
            tc.tile_pool(name="zpsum", bufs=2, space="PSUM") as zpsum,
            tc.tile_pool(name="opsum", bufs=2, space="PSUM") as opsum,
        ):
            # resident constants; ordered so slab 0's z can start ASAP.
            # GPSIMD's DMA queue issues in 25ns (vs ~550 on SP), so the
            # first-z inputs split across both queues for minimum latency.
            w_sb = singles.tile([128, N_CHUNKS, 128], f16)
            nc.gpsimd.dma_start(out=w_sb[:, 0, :], in_=wt_d[:, 0, :])
            xt2 = singles.tile([128, B_CORE], f16)
            nc.sync.dma_start(out=xt2[:, 0:256], in_=xt2_d[:, 0:256])
            nc.sync.dma_start(out=xt2[:, 256:1024], in_=xt2_d[:, 256:1024])
            nc.sync.dma_start(out=w_sb[:, 1:, :], in_=wt_d[:, 1:, :])
            ones0 = singles.tile([128, 8, 1], f16)
            nc.vector.memset(ones0, 1.0)
            sigwarm = singles.tile([1, 1], f16)
            nc.vector.memset(sigwarm, 0.0)
            nc.scalar.activation(sigwarm, sigwarm, SIG, bias=0.0, scale=1.0)
            m_sb = singles.tile([128, N_CHUNKS, 16], f16)
            nc.sync.dma_start(out=m_sb, in_=mt_d[:, :, :])
            # remaining slabs arrive via the (otherwise idle) GPSIMD DMA
            # queue so they do not queue behind the SP constants
            for i in range(1, N_SLABS):
                sl = bass.ts(i, SLAB)
                nc.gpsimd.dma_start(out=xt2[:, sl], in_=xt2_d[:, sl])

            state = {}

            def emit_zacts(sl, c0, c1, split=False):
                """z matmuls + one sigmoid for flat gate cols [c0, c1)."""
                gt = state[("gt", sl)]
                with tc.high_priority():
                    z = zpsum.tile([128, c1 - c0], f32, tag="z",
                                   name=f"z{sl}_{c0}")
                    for j in range(c0 // 512, c1 // 512):
                        ch, h = divmod(j, 2)
                        nc.tensor.matmul(
                            z[:, j * 512 - c0: j * 512 - c0 + 512],
                            lhsT=w_sb[:, ch, :],
                            rhs=xt2[:, sl * SLAB + h * 512:
                                    sl * SLAB + (h + 1) * 512],
                            start=True,
                            stop=True,
                        )
                    if split:
                        mid = (c0 + c1) // 2
                        nc.scalar.activation(gt[:, c0:mid], z[:, 0:mid - c0],
                                             SIG, bias=0.0, scale=1.0)
                        nc.scalar.activation(gt[:, mid:c1],
                                             z[:, mid - c0:c1 - c0],
                                             SIG, bias=0.0, scale=1.0)
                    else:
                        nc.scalar.activation(gt[:, c0:c1], z, SIG,
                                             bias=0.0, scale=1.0)

            def emit_shallow(sl):
                """gt chunk0 -> batch-major, cascade levels 0-6 -> p7t."""
                gt = state[("gt", sl)]
                gsh = work.tile([128, 8, 128], f16, tag="gsh", name=f"gsh{sl}")
                for g in range(8):
                    nc.sync.dma_start(
                        out=gsh[:, g, :],
                        in_=gt[:, g * 128: (g + 1) * 128],
                        transpose=True,
                    )
                prev = ones0[:, 0:8, :]
                for d in range(7):
                    n = 1 << d
                    cur = work.tile([128, 8, 2 * n], f16, tag=f"pb{d + 1}",
                                    name=f"pb{d + 1}_{sl}")
                    gl = gsh[:, :, n - 1: 2 * n - 1]
                    nc.vector.tensor_mul(cur[:, :, 0:n], prev, gl)
                    nc.vector.tensor_sub(cur[:, :, n: 2 * n], prev,
                                         cur[:, :, 0:n])
                    prev = cur
                p7t = work.tile([128, SLAB], f16, tag="p7t", name=f"p7t{sl}")
                for g in range(8):
                    nc.sync.dma_start(
                        out=p7t[:, g * 128: (g + 1) * 128],
                        in_=prev[:, g, :],
                        transpose=True,
                    )
                state[("p7t", sl)] = p7t

            def prefetch(nxt):
                state[("gt", nxt)] = gtpool.tile([128, GFLAT], f16, tag="gt",
                                                 name=f"gt{nxt}")
                emit_zacts(nxt, 0, SLAB)
                emit_shallow(nxt)

            def emit_slab(sl, last):
                gt = state[("gt", sl)]
                op = opsum.tile([128, 8, 16], f32, tag="op", name=f"op{sl}")
                osb = work.tile([128, 8, 16], f32, tag="osb", name=f"osb{sl}")
                F = {}

                def gch(ch, h0=0, h1=SLAB):
                    return gt[:, ch * SLAB + h0: ch * SLAB + h1]

                def ftile(name):
                    t = fwork.tile([128, SLAB], f16, tag=name,
                                   name=f"{name}_{sl}")
                    F[name] = t
                    return t

                def folds(name, m_idx, start=False, stop=False, bsubs=None):
                    # start=True only on the very first matmul of this op
                    # tile: it marks the whole 2KB PSUM zero-region pending,
                    # so each chain's first touch initializes and later
                    # touches accumulate. A second start would clobber
                    # sibling chains sharing the bank.
                    src = F[name]
                    for i, b in enumerate(range(8) if bsubs is None else bsubs):
                        nc.tensor.matmul(
                            op[:, b, 0:N_CLASS],
                            lhsT=src[:, b * 128: (b + 1) * 128],
                            rhs=m_sb[:, m_idx, 0:N_CLASS],
                            start=start and i == 0,
                            stop=stop,
                            skip_group_check=True,
                        )

                # sigmoid tiles for chunks 1-7: 4x1536 + 1x1024 flat columns
                bounds = [1024, 2560, 4096, 5632, 7168, 8192]
                for u in range(5):
                    emit_zacts(sl, bounds[u], bounds[u + 1],
                               split=(last and u == 4))
                    if u == 0:
                        # prefetch chunk-0 gates + shallow chains two windows
                        # ahead: the gsh/cascade/p7t chain has ~5us of
                        # latency (two DMA-semaphore hops + serial cascade)
                        if sl + 2 < N_SLABS:
                            prefetch(sl + 2)
                        # ch1 complete
                        p7t = state[("p7t", sl)]
                        l8 = ftile("l8")
                        nc.vector.tensor_mul(l8, p7t, gch(1))
                        r8 = ftile("r8")
                        nc.vector.tensor_sub(r8, p7t, l8)
                    elif u == 1:
                        # ch2 + ch3 complete
                        l9a = ftile("l9a")
                        nc.vector.tensor_mul(l9a, F["l8"], gch(2))
                        r9a = ftile("r9a")
                        nc.vector.tensor_sub(r9a, F["l8"], l9a)
                        l9b = ftile("l9b")
                        nc.gpsimd.tensor_mul(l9b, F["r8"], gch(3))
                        # r9b on DVE: it feeds q3, the tail-critical chain
                        r9b = ftile("r9b")
                        nc.vector.tensor_sub(r9b, F["r8"], l9b)
                        folds("l9a", 0, start=True)
                        folds("r9a", 1)
                        folds("l9b", 2)
                        folds("r9b", 3)
                    elif u == 2:
                        # ch4 complete
                        q0 = ftile("q0")
                        nc.gpsimd.tensor_mul(q0, F["l9a"], gch(4))
                        folds("q0", 4)
                    elif u == 3:
                        # ch5 + ch6 complete
                        q1 = ftile("q1")
                        nc.gpsimd.tensor_mul(q1, F["l9b"], gch(5))
                        q2 = ftile("q2")
                        nc.vector.tensor_mul(q2, F["r9a"], gch(6))
                        folds("q1", 5)
                        folds("q2", 6)
                    elif u == 4:
                        # ch7 complete
                        q3 = ftile("q3")
                        if last:
                            for h in range(2):
                                hs = bass.ts(h, 512)
                                nc.vector.tensor_mul(
                                    q3[:, hs], F["r9b"][:, hs],
                                    gch(7, h * 512, (h + 1) * 512))
                                folds("q3", 7, stop=True,
                                      bsubs=range(4 * h, 4 * h + 4))
                                bs = slice(4 * h, 4 * h + 4)
                                nc.vector.tensor_copy(
                                    osb[:, bs, :], op[:, bs, :])
                                nc.gpsimd.dma_start(
                                    out=out_d[:, sl, bs, :],
                                    in_=osb[:, bs, :],
                                )
                        else:
                            nc.vector.tensor_mul(q3, F["r9b"], gch(7))
                            folds("q3", 7, stop=True)
                            nc.vector.tensor_copy(osb, op)
                            nc.gpsimd.dma_start(
                                out=out_d[:, sl, :, :], in_=osb[:, :, :])

            # slab 0 chunk-0 in a 256 + 768 piece pair so the very first
            # sigmoid fires as soon as the first 256 x-columns land
            state[("gt", 0)] = gtpool.tile([128, GFLAT], f16,
                                           tag="gt", name="gt0")
            gt0 = state[("gt", 0)]
            with tc.high_priority():
                za = zpsum.tile([128, 256], f32, tag="z", name="z0_head")
                nc.tensor.matmul(za, lhsT=w_sb[:, 0, :], rhs=xt2[:, 0:256],
                                 start=True, stop=True)
                nc.scalar.activation(gt0[:, 0:256], za, SIG,
                                     bias=0.0, scale=1.0)
                zb = zpsum.tile([128, 768], f32, tag="z", name="z0_head2")
                # one start per PSUM bank; within a started bank the first
                # touch initializes (bytes are pending-zero)
                for i, st in ((0, True), (1, False), (2, True)):
                    nc.tensor.matmul(
                        zb[:, i * 256: (i + 1) * 256],
                        lhsT=w_sb[:, 0, :],
                        rhs=xt2[:, 256 + i * 256: 512 + i * 256],
                        start=st,
                        stop=True,
                    )
                nc.scalar.activation(gt0[:, 256:1024], zb, SIG,
                                     bias=0.0, scale=1.0)
            emit_shallow(0)
            state[("gt", 1)] = gtpool.tile([128, GFLAT], f16,
                                           tag="gt", name="gt1")
            emit_zacts(1, 0, SLAB)
            emit_shallow(1)
            for sl in range(N_SLABS):
                emit_slab(sl, last=(sl == N_SLABS - 1))

    nc.finalize()
    return nc


def _get_program():
    if "nc" not in _CACHE:
        _CACHE["nc"] = _build_program()
    return _CACHE["nc"]


def kernel(x, c, s, leaf_logits, dims, max_depth):
    from concourse.bass_utils import run_bass_kernel_spmd

    assert int(max_depth) == MAX_DEPTH
    x = np.asarray(x, dtype=F32)
    c = np.asarray(c, dtype=F32)
    s = np.asarray(s, dtype=F32)
    leaf_logits = np.asarray(leaf_logits, dtype=F32)
    dims = np.asarray(dims)

    W, lo_feats, M = _build_constants(c, s, dims, leaf_logits)
    wt = np.ascontiguousarray(W.transpose(1, 0, 2))            # [128, 8, 128]
    mt = np.ascontiguousarray(M.transpose(1, 0, 2))            # [128, 8, 16]

    in_maps = []
    for core in range(N_CORES):
        xc = x[core * B_CORE: (core + 1) * B_CORE]             # [8192, 64]
        xT = np.ascontiguousarray(xc.T).astype(F32)            # [64, 8192]
        x_hi = xT.astype(F16)
        x_lo = (xT - x_hi.astype(F32)).astype(F16)
        xt2 = np.empty((128, B_CORE), dtype=F16)               # [128, 8192]
        xt2[:IN_DIM] = x_hi
        xt2[IN_DIM:126] = x_lo[lo_feats]
        xt2[126:] = 1.0                                        # bias rows
        in_maps.append({"xt2": xt2, "wt": wt, "mt": mt})

    _CACHE["in_maps"] = in_maps
    nc = _get_program()
    res = run_bass_kernel_spmd(nc, in_maps, core_ids=list(range(N_CORES)))

    out = np.empty((B_FULL, N_CLASS), dtype=F32)
    for core in range(N_CORES):
        outt = res.results[core]["outt"]                # [128, 8, 8, 16] f32
        arr = outt[:, :, :, :N_CLASS].transpose(1, 2, 0, 3).reshape(
            B_CORE, N_CLASS)
        out[core * B_CORE: (core + 1) * B_CORE] = arr.astype(F32)
    return out


# revision 31
# speedup vs baseline: 1.0059x; 1.0002x over previous
"""Trainium2 Bass kernel for nn_DFPT_Node (soft binary decision tree).

Full inputs in, full output out; internally data-parallel over 8 NeuronCores
(batch sharded 65536 -> 8 x 8192). Tree params (c, s, dims, leaf_logits) are
baked into compiled constants on the host:

  gate:  g = sigmoid(-4 (x[:,dims] - c)/|s|) = sigmoid(a*x + b),
         a = -4/|s| as a scaled one-hot matmul (fp16 hi/lo split of x for
         precision); b rides inside W as two fp16 rows against constant-1
         xt2 rows, so the sigmoid needs no per-chunk ACT bias and one ACT
         op can span chunk boundaries.
  tree:  levels 0-6 batch-major (batch on partitions), levels 7-9 node-major
         (nodes on partitions, batch on free dim) in block (bit-reversed)
         leaf order; level 9 folded into the output matmul in q-basis:
         out = sum_t q_t @ A_t + p9_t @ B_t, accumulated in PSUM via
         batch-major flipped matmuls (lhsT = F batch tile, rhs = M chunk
         [128,10] -> ~free on the PE cost model).

The kernel is ACT(sigmoid)-bound: 8.4M sigmoid elements/core at
0.83ns/col/128 lanes ~= 54.6us processing. The schedule keeps ACT saturated:
z tiles of 1536 columns (3 PSUM banks, double buffered, next to a 2-bank op
accumulator) amortize ACT init; chunk-0 gates + the shallow cascade prefetch
two windows ahead (the gsh/cascade/p7t chain has ~5us of DMA-semaphore
latency); each later chunk's deep products and fold matmuls follow its
sigmoid within the window (lag-0), so only head latency and a ~3.3us
post-sigmoid drain remain.

Output leaves the device batch-major as outt [128, 8, 8, 16] f32 per core.
"""

import numpy as np

B_FULL = 65536
IN_DIM = 64
N_CLASS = 10
MAX_DEPTH = 10
N_CORES = 8
B_CORE = B_FULL // N_CORES      # 8192
SLAB = 1024                     # batch columns processed per slab
N_SLABS = B_CORE // SLAB        # 8
N_CHUNKS = 8                    # node-major chunks of 128 nodes

F16 = np.float16
F32 = np.float32

_CACHE = {}


def _build_tree_layout():
    """pos[d][i] = reference position within level d of block-order index i."""
    pos = [np.array([0], dtype=np.int64)]
    for _ in range(MAX_DEPTH):
        p = pos[-1]
        pos.append(np.concatenate([2 * p, 2 * p + 1]))
    return pos


def _build_constants(c, s, dims, leaf_logits):
    """W chunks [8,128,128] f16, dropped-lo features, M chunks [8,128,16].

    The bias rides inside W as two fp16 rows (126: fp16(b), 127: fp16
    residual) against constant-1.0 xt2 rows, freeing the ACT engine from a
    per-chunk bias operand. To make room, the two features with the
    smallest max|a| lose their x_lo row (fp16-only x for those features;
    validated ~1e-7 effect on output error for this tree).
    """
    pos = _build_tree_layout()
    chunk_nodes = -np.ones((N_CHUNKS, 128), dtype=np.int64)
    for d in range(7):
        base = (1 << d) - 1
        chunk_nodes[0, base: base + (1 << d)] = base + pos[d]
    chunk_nodes[1, :] = 127 + pos[7]
    lvl8 = 255 + pos[8]
    chunk_nodes[2, :] = lvl8[:128]
    chunk_nodes[3, :] = lvl8[128:]
    lvl9 = 511 + pos[9]
    for t in range(4):
        chunk_nodes[4 + t, :] = lvl9[128 * t: 128 * (t + 1)]

    a64 = -4.0 / np.abs(s.astype(np.float64))
    a16 = a64.astype(F16)
    b64 = -a16.astype(np.float64) * c.astype(np.float64)
    b1 = b64.astype(F16)
    b2 = (b64 - b1.astype(np.float64)).astype(F16)

    dims = dims.astype(np.int64)
    feat_max = np.zeros(IN_DIM)
    np.maximum.at(feat_max, dims, np.abs(a64))
    drop = np.argsort(feat_max)[:2]
    lo_feats = np.array([k for k in range(IN_DIM) if k not in drop])
    lo_row = -np.ones(IN_DIM, dtype=np.int64)
    lo_row[lo_feats] = IN_DIM + np.arange(IN_DIM - 2)

    W = np.zeros((N_CHUNKS, 128, 128), dtype=F16)
    ch_idx, col_idx = np.nonzero(chunk_nodes >= 0)
    g_idx = chunk_nodes[ch_idx, col_idx]
    d_idx = dims[g_idx]
    W[ch_idx, d_idx, col_idx] = a16[g_idx]
    has_lo = lo_row[d_idx] >= 0
    W[ch_idx[has_lo], lo_row[d_idx[has_lo]], col_idx[has_lo]] = (
        a16[g_idx[has_lo]])
    W[ch_idx, 126, col_idx] = b1[g_idx]
    W[ch_idx, 127, col_idx] = b2[g_idx]

    L_my = leaf_logits[pos[MAX_DEPTH]].astype(np.float64)  # [1024, 10]
    A = L_my[:512] - L_my[512:]
    Bm = L_my[512:]
    # M chunk i pairs with F chunk i in fold order
    # [l9a, r9a, l9b, r9b, q0, q1, q2, q3]; p9 block order is
    # [l9a, l9b, r9a, r9b] over Bm quarters, q block order over A quarters.
    Mlist = [Bm[0:128], Bm[256:384], Bm[128:256], Bm[384:512],
             A[0:128], A[128:256], A[256:384], A[384:512]]
    M = np.zeros((N_CHUNKS, 128, 16), dtype=F16)
    for i, m in enumerate(Mlist):
        M[i, :, :N_CLASS] = m.astype(F16)
    return W, lo_feats, M


def _build_program():
    import concourse.bass as bass
    import concourse.tile as tile
    from concourse import bacc, mybir

    f16 = mybir.dt.float16
    f32 = mybir.dt.float32
    SIG = mybir.ActivationFunctionType.Sigmoid

    nc = bacc.Bacc("TRN2", target_bir_lowering=False)
    xt2_d = nc.dram_tensor("xt2", [128, B_CORE], f16, kind="ExternalInput")
    wt_d = nc.dram_tensor("wt", [128, N_CHUNKS, 128], f16, kind="ExternalInput")
    mt_d = nc.dram_tensor("mt", [128, N_CHUNKS, 16], f16, kind="ExternalInput")
    out_d = nc.dram_tensor("outt", [128, N_SLABS, 8, 16], f32,
                           kind="ExternalOutput")

    # flat per-slab gate layout: gt[:, ch*1024 + col], so sigmoid tiles can
    # span chunk boundaries (the bias lives inside the W matmul)
    GFLAT = N_CHUNKS * SLAB

    with tile.TileContext(nc) as tc:
        with (
            tc.tile_pool(name="singles", bufs=1) as singles,
            tc.tile_pool(name="work", bufs=3) as work,
            tc.tile_pool(name="gtpool", bufs=4) as gtpool,
            tc.tile_pool(name="fwork", bufs=2) as fwork,
            tc.tile_pool(name="zpsum", bufs=2, space="PSUM") as zpsum,
            tc.tile_pool(name="opsum", bufs=2, space="PSUM") as opsum,
        ):
            # resident constants; ordered so slab 0's z can start ASAP.
            # GPSIMD's DMA queue issues in 25ns (vs ~550 on SP), so the
            # first-z inputs split across both queues for minimum latency.
            w_sb = singles.tile([128, N_CHUNKS, 128], f16)
            nc.gpsimd.dma_start(out=w_sb[:, 0, :], in_=wt_d[:, 0, :])
            xt2 = singles.tile([128, B_CORE], f16)
            nc.sync.dma_start(out=xt2[:, 0:512], in_=xt2_d[:, 0:512])
            nc.sync.dma_start(out=xt2[:, 512:1024], in_=xt2_d[:, 512:1024])
            nc.sync.dma_start(out=w_sb[:, 1:, :], in_=wt_d[:, 1:, :])
            ones0 = singles.tile([128, 8, 1], f16)
            nc.vector.memset(ones0, 1.0)
            sigwarm = singles.tile([1, 1], f16)
            nc.vector.memset(sigwarm, 0.0)
            nc.scalar.activation(sigwarm, sigwarm, SIG, bias=0.0, scale=1.0)
            m_sb = singles.tile([128, N_CHUNKS, 16], f16)
            nc.sync.dma_start(out=m_sb, in_=mt_d[:, :, :])
            # remaining slabs arrive via the (otherwise idle) GPSIMD DMA
            # queue so they do not queue behind the SP constants
            for i in range(1, N_SLABS):
                sl = bass.ts(i, SLAB)
                nc.gpsimd.dma_start(out=xt2[:, sl], in_=xt2_d[:, sl])

            state = {}

            def emit_zacts(sl, c0, c1, split=False):
                """z matmuls + one sigmoid for flat gate cols [c0, c1)."""
                gt = state[("gt", sl)]
                with tc.high_priority():
                    z = zpsum.tile([128, c1 - c0], f32, tag="z",
                                   name=f"z{sl}_{c0}")
                    for j in range(c0 // 512, c1 // 512):
                        ch, h = divmod(j, 2)
                        nc.tensor.matmul(
                            z[:, j * 512 - c0: j * 512 - c0 + 512],
                            lhsT=w_sb[:, ch, :],
                            rhs=xt2[:, sl * SLAB + h * 512:
                                    sl * SLAB + (h + 1) * 512],
                            start=True,
                            stop=True,
                        )
                    if split:
                        mid = (c0 + c1) // 2
                        nc.scalar.activation(gt[:, c0:mid], z[:, 0:mid - c0],
                                             SIG, bias=0.0, scale=1.0)
                        nc.scalar.activation(gt[:, mid:c1],
                                             z[:, mid - c0:c1 - c0],
                                             SIG, bias=0.0, scale=1.0)
                    else:
                        nc.scalar.activation(gt[:, c0:c1], z, SIG,
                                             bias=0.0, scale=1.0)

            def emit_shallow(sl):
                """gt chunk0 -> batch-major, cascade levels 0-6 -> p7t."""
                gt = state[("gt", sl)]
                gsh = work.tile([128, 8, 128], f16, tag="gsh", name=f"gsh{sl}")
                for g in range(8):
                    nc.sync.dma_start(
                        out=gsh[:, g, :],
                        in_=gt[:, g * 128: (g + 1) * 128],
                        transpose=True,
                    )
                prev = ones0[:, 0:8, :]
                for d in range(7):
                    n = 1 << d
                    cur = work.tile([128, 8, 2 * n], f16, tag=f"pb{d + 1}",
                                    name=f"pb{d + 1}_{sl}")
                    gl = gsh[:, :, n - 1: 2 * n - 1]
                    nc.vector.tensor_mul(cur[:, :, 0:n], prev, gl)
                    nc.vector.tensor_sub(cur[:, :, n: 2 * n], prev,
                                         cur[:, :, 0:n])
                    prev = cur
                p7t = work.tile([128, SLAB], f16, tag="p7t", name=f"p7t{sl}")
                for g in range(8):
                    nc.sync.dma_start(
                        out=p7t[:, g * 128: (g + 1) * 128],
                        in_=prev[:, g, :],
                        transpose=True,
                    )
                state[("p7t", sl)] = p7t

            def prefetch(nxt):
                state[("gt", nxt)] = gtpool.tile([128, GFLAT], f16, tag="gt",
                                                 name=f"gt{nxt}")
                emit_zacts(nxt, 0, SLAB)
                emit_shallow(nxt)

            def emit_slab(sl, last):
                gt = state[("gt", sl)]
                op = opsum.tile([128, 8, 16], f32, tag="op", name=f"op{sl}")
                osb = work.tile([128, 8, 16], f32, tag="osb", name=f"osb{sl}")
                F = {}

                def gch(ch, h0=0, h1=SLAB):
                    return gt[:, ch * SLAB + h0: ch * SLAB + h1]

                def ftile(name):
                    t = fwork.tile([128, SLAB], f16, tag=name,
                                   name=f"{name}_{sl}")
                    F[name] = t
                    return t

                def folds(name, m_idx, start=False, stop=False, bsubs=None):
                    # start=True only on the very first matmul of this op
                    # tile: it marks the whole 2KB PSUM zero-region pending,
                    # so each chain's first touch initializes and later
                    # touches accumulate. A second start would clobber
                    # sibling chains sharing the bank.
                    src = F[name]
                    for i, b in enumerate(range(8) if bsubs is None else bsubs):
                        nc.tensor.matmul(
                            op[:, b, 0:N_CLASS],
                            lhsT=src[:, b * 128: (b + 1) * 128],
                            rhs=m_sb[:, m_idx, 0:N_CLASS],
                            start=start and i == 0,
                            stop=stop,
                            skip_group_check=True,
                        )

                # sigmoid tiles for chunks 1-7: 4x1536 + 1x1024 flat columns
                bounds = [1024, 2560, 4096, 5632, 7168, 8192]
                for u in range(5):
                    emit_zacts(sl, bounds[u], bounds[u + 1],
                               split=(last and u == 4))
                    if u == 0:
                        # prefetch chunk-0 gates + shallow chains two windows
                        # ahead: the gsh/cascade/p7t chain has ~5us of
                        # latency (two DMA-semaphore hops + serial cascade)
                        if sl + 2 < N_SLABS:
                            prefetch(sl + 2)
                        # ch1 complete
                        p7t = state[("p7t", sl)]
                        l8 = ftile("l8")
                        nc.vector.tensor_mul(l8, p7t, gch(1))
                        r8 = ftile("r8")
                        nc.vector.tensor_sub(r8, p7t, l8)
                    elif u == 1:
                        # ch2 + ch3 complete
                        l9a = ftile("l9a")
                        nc.vector.tensor_mul(l9a, F["l8"], gch(2))
                        r9a = ftile("r9a")
                        nc.vector.tensor_sub(r9a, F["l8"], l9a)
                        l9b = ftile("l9b")
                        nc.gpsimd.tensor_mul(l9b, F["r8"], gch(3))
                        # r9b on DVE: it feeds q3, the tail-critical chain
                        r9b = ftile("r9b")
                        nc.vector.tensor_sub(r9b, F["r8"], l9b)
                        folds("l9a", 0, start=True)
                        folds("r9a", 1)
                        folds("l9b", 2)
                        folds("r9b", 3)
                    elif u == 2:
                        # ch4 complete
                        q0 = ftile("q0")
                        nc.gpsimd.tensor_mul(q0, F["l9a"], gch(4))
                        folds("q0", 4)
                    elif u == 3:
                        # ch5 + ch6 complete
                        q1 = ftile("q1")
                        nc.gpsimd.tensor_mul(q1, F["l9b"], gch(5))
                        q2 = ftile("q2")
                        nc.vector.tensor_mul(q2, F["r9a"], gch(6))
                        folds("q1", 5)
                        folds("q2", 6)
                    elif u == 4:
                        # ch7 complete
                        q3 = ftile("q3")
                        if last:
                            for h in range(2):
                                hs = bass.ts(h, 512)
                                nc.vector.tensor_mul(
                                    q3[:, hs], F["r9b"][:, hs],
                                    gch(7, h * 512, (h + 1) * 512))
                                folds("q3", 7, stop=True,
                                      bsubs=range(4 * h, 4 * h + 4))
                                bs = slice(4 * h, 4 * h + 4)
                                nc.vector.tensor_copy(
                                    osb[:, bs, :], op[:, bs, :])
                                nc.gpsimd.dma_start(
                                    out=out_d[:, sl, bs, :],
                                    in_=osb[:, bs, :],
                                )
                        else:
                            nc.vector.tensor_mul(q3, F["r9b"], gch(7))
                            folds("q3", 7, stop=True)
                            nc.vector.tensor_copy(osb, op)
                            nc.gpsimd.dma_start(
                                out=out_d[:, sl, :, :], in_=osb[:, :, :])

            # slab 0 chunk-0 as two 512-col piece pairs: the first sigmoid
            # fires as soon as the first half of x lands, and the second
            # half's z fill hides under the first sigmoid
            state[("gt", 0)] = gtpool.tile([128, GFLAT], f16,
                                           tag="gt", name="gt0")
            gt0 = state[("gt", 0)]
            with tc.high_priority():
                for h in range(2):
                    hs = bass.ts(h, 512)
                    zh = zpsum.tile([128, 512], f32, tag="z",
                                    name=f"z0_head{h}")
                    nc.tensor.matmul(zh, lhsT=w_sb[:, 0, :], rhs=xt2[:, hs],
                                     start=True, stop=True)
                    nc.scalar.activation(gt0[:, hs], zh, SIG,
                                         bias=0.0, scale=1.0)
            emit_shallow(0)
            state[("gt", 1)] = gtpool.tile([128, GFLAT], f16,
                                           tag="gt", name="gt1")
            emit_zacts(1, 0, SLAB)
            emit_shallow(1)
            for sl in range(N_SLABS):
                emit_slab(sl, last=(sl == N_SLABS - 1))

    nc.finalize()
    return nc


def _get_program():
    if "nc" not in _CACHE:
        _CACHE["nc"] = _build_program()
    return _CACHE["nc"]


def kernel(x, c, s, leaf_logits, dims, max_depth):
    from concourse.bass_utils import run_bass_kernel_spmd

    assert int(max_depth) == MAX_DEPTH
    x = np.asarray(x, dtype=F32)
    c = np.asarray(c, dtype=F32)
    s = np.asarray(s, dtype=F32)
    leaf_logits = np.asarray(leaf_logits, dtype=F32)
    dims = np.asarray(dims)

    W, lo_feats, M = _build_constants(c, s, dims, leaf_logits)
    wt = np.ascontiguousarray(W.transpose(1, 0, 2))            # [128, 8, 128]
    mt = np.ascontiguousarray(M.transpose(1, 0, 2))            # [128, 8, 16]

    in_maps = []
    for core in range(N_CORES):
        xc = x[core * B_CORE: (core + 1) * B_CORE]             # [8192, 64]
        xT = np.ascontiguousarray(xc.T).astype(F32)            # [64, 8192]
        x_hi = xT.astype(F16)
        x_lo = (xT - x_hi.astype(F32)).astype(F16)
        xt2 = np.empty((128, B_CORE), dtype=F16)               # [128, 8192]
        xt2[:IN_DIM] = x_hi
        xt2[IN_DIM:126] = x_lo[lo_feats]
        xt2[126:] = 1.0                                        # bias rows
        in_maps.append({"xt2": xt2, "wt": wt, "mt": mt})

    _CACHE["in_maps"] = in_maps
    nc = _get_program()
    res = run_bass_kernel_spmd(nc, in_maps, core_ids=list(range(N_CORES)))

    out = np.empty((B_FULL, N_CLASS), dtype=F32)
    for core in range(N_CORES):
        outt = res.results[core]["outt"]                # [128, 8, 8, 16] f32
        arr = outt[:, :, :, :N_CLASS].transpose(1, 2, 0, 3).reshape(
            B_CORE, N_CLASS)
        out[core * B_CORE: (core + 1) * B_CORE] = arr.astype(F32)
    return out


# revision 32
# speedup vs baseline: 1.0082x; 1.0024x over previous
"""Trainium2 Bass kernel for nn_DFPT_Node (soft binary decision tree).

Full inputs in, full output out; internally data-parallel over 8 NeuronCores
(batch sharded 65536 -> 8 x 8192). Tree params (c, s, dims, leaf_logits) are
baked into compiled constants on the host:

  gate:  g = sigmoid(-4 (x[:,dims] - c)/|s|) = sigmoid(a*x + b),
         a = -4/|s| as a scaled one-hot matmul (fp16 hi/lo split of x for
         precision); b rides inside W as two fp16 rows against constant-1
         xt2 rows, so the sigmoid needs no per-chunk ACT bias and one ACT
         op can span chunk boundaries.
  tree:  levels 0-6 batch-major (batch on partitions), levels 7-9 node-major
         (nodes on partitions, batch on free dim) in block (bit-reversed)
         leaf order; level 9 folded into the output matmul in q-basis:
         out = sum_t q_t @ A_t + p9_t @ B_t, accumulated in PSUM via
         batch-major flipped matmuls (lhsT = F batch tile, rhs = M chunk
         [128,10] -> ~free on the PE cost model).

The kernel is ACT(sigmoid)-bound: 8.4M sigmoid elements/core at
0.83ns/col/128 lanes ~= 54.6us processing. The schedule keeps ACT saturated:
z tiles of 1536 columns (3 PSUM banks, double buffered, next to a 2-bank op
accumulator) amortize ACT init; chunk-0 gates + the shallow cascade prefetch
two windows ahead (the gsh/cascade/p7t chain has ~5us of DMA-semaphore
latency); each later chunk's deep products and fold matmuls follow its
sigmoid within the window (lag-0), so only head latency and a ~3.3us
post-sigmoid drain remain.

Output leaves the device batch-major as outt [128, 8, 8, 16] f32 per core.
"""

import numpy as np

B_FULL = 65536
IN_DIM = 64
N_CLASS = 10
MAX_DEPTH = 10
N_CORES = 8
B_CORE = B_FULL // N_CORES      # 8192
SLAB = 1024                     # batch columns processed per slab
N_SLABS = B_CORE // SLAB        # 8
N_CHUNKS = 8                    # node-major chunks of 128 nodes

F16 = np.float16
F32 = np.float32

_CACHE = {}


def _build_tree_layout():
    """pos[d][i] = reference position within level d of block-order index i."""
    pos = [np.array([0], dtype=np.int64)]
    for _ in range(MAX_DEPTH):
        p = pos[-1]
        pos.append(np.concatenate([2 * p, 2 * p + 1]))
    return pos


def _build_constants(c, s, dims, leaf_logits):
    """W chunks [8,128,128] f16, dropped-lo features, M chunks [8,128,16].

    The bias rides inside W as two fp16 rows (126: fp16(b), 127: fp16
    residual) against constant-1.0 xt2 rows, freeing the ACT engine from a
    per-chunk bias operand. To make room, the two features with the
    smallest max|a| lose their x_lo row (fp16-only x for those features;
    validated ~1e-7 effect on output error for this tree).
    """
    pos = _build_tree_layout()
    chunk_nodes = -np.ones((N_CHUNKS, 128), dtype=np.int64)
    for d in range(7):
        base = (1 << d) - 1
        chunk_nodes[0, base: base + (1 << d)] = base + pos[d]
    chunk_nodes[1, :] = 127 + pos[7]
    lvl8 = 255 + pos[8]
    chunk_nodes[2, :] = lvl8[:128]
    chunk_nodes[3, :] = lvl8[128:]
    lvl9 = 511 + pos[9]
    for t in range(4):
        chunk_nodes[4 + t, :] = lvl9[128 * t: 128 * (t + 1)]

    a64 = -4.0 / np.abs(s.astype(np.float64))
    a16 = a64.astype(F16)
    b64 = -a16.astype(np.float64) * c.astype(np.float64)
    b1 = b64.astype(F16)
    b2 = (b64 - b1.astype(np.float64)).astype(F16)

    dims = dims.astype(np.int64)
    feat_max = np.zeros(IN_DIM)
    np.maximum.at(feat_max, dims, np.abs(a64))
    drop = np.argsort(feat_max)[:2]
    lo_feats = np.array([k for k in range(IN_DIM) if k not in drop])
    lo_row = -np.ones(IN_DIM, dtype=np.int64)
    lo_row[lo_feats] = IN_DIM + np.arange(IN_DIM - 2)

    W = np.zeros((N_CHUNKS, 128, 128), dtype=F16)
    ch_idx, col_idx = np.nonzero(chunk_nodes >= 0)
    g_idx = chunk_nodes[ch_idx, col_idx]
    d_idx = dims[g_idx]
    W[ch_idx, d_idx, col_idx] = a16[g_idx]
    has_lo = lo_row[d_idx] >= 0
    W[ch_idx[has_lo], lo_row[d_idx[has_lo]], col_idx[has_lo]] = (
        a16[g_idx[has_lo]])
    W[ch_idx, 126, col_idx] = b1[g_idx]
    W[ch_idx, 127, col_idx] = b2[g_idx]

    L_my = leaf_logits[pos[MAX_DEPTH]].astype(np.float64)  # [1024, 10]
    A = L_my[:512] - L_my[512:]
    Bm = L_my[512:]
    # M chunk i pairs with F chunk i in fold order
    # [l9a, r9a, l9b, r9b, q0, q1, q2, q3]; p9 block order is
    # [l9a, l9b, r9a, r9b] over Bm quarters, q block order over A quarters.
    Mlist = [Bm[0:128], Bm[256:384], Bm[128:256], Bm[384:512],
             A[0:128], A[128:256], A[256:384], A[384:512]]
    M = np.zeros((N_CHUNKS, 128, 16), dtype=F16)
    for i, m in enumerate(Mlist):
        M[i, :, :N_CLASS] = m.astype(F16)
    return W, lo_feats, M


def _build_program():
    import concourse.bass as bass
    import concourse.tile as tile
    from concourse import bacc, mybir

    f16 = mybir.dt.float16
    f32 = mybir.dt.float32
    SIG = mybir.ActivationFunctionType.Sigmoid

    nc = bacc.Bacc("TRN2", target_bir_lowering=False)
    xt2_d = nc.dram_tensor("xt2", [128, B_CORE], f16, kind="ExternalInput")
    wt_d = nc.dram_tensor("wt", [128, N_CHUNKS, 128], f16, kind="ExternalInput")
    mt_d = nc.dram_tensor("mt", [128, N_CHUNKS, 16], f16, kind="ExternalInput")
    out_d = nc.dram_tensor("outt", [128, N_SLABS, 8, 16], f32,
                           kind="ExternalOutput")

    # flat per-slab gate layout: gt[:, ch*1024 + col], so sigmoid tiles can
    # span chunk boundaries (the bias lives inside the W matmul)
    GFLAT = N_CHUNKS * SLAB

    with tile.TileContext(nc) as tc:
        with (
            tc.tile_pool(name="singles", bufs=1) as singles,
            tc.tile_pool(name="work", bufs=3) as work,
            tc.tile_pool(name="gtpool", bufs=4) as gtpool,
            tc.tile_pool(name="fwork", bufs=2) as fwork,
            tc.tile_pool(name="zpsum", bufs=2, space="PSUM") as zpsum,
            tc.tile_pool(name="opsum", bufs=2, space="PSUM") as opsum,
        ):
            # resident constants; ordered so slab 0's z can start ASAP.
            # GPSIMD's DMA queue issues in 25ns (vs ~550 on SP), so the
            # first-z inputs split across both queues for minimum latency.
            w_sb = singles.tile([128, N_CHUNKS, 128], f16)
            nc.gpsimd.dma_start(out=w_sb[:, 0, :], in_=wt_d[:, 0, :])
            xt2 = singles.tile([128, B_CORE], f16)
            nc.sync.dma_start(out=xt2[:, 0:512], in_=xt2_d[:, 0:512])
            nc.sync.dma_start(out=xt2[:, 512:1024], in_=xt2_d[:, 512:1024])
            nc.sync.dma_start(out=w_sb[:, 1:, :], in_=wt_d[:, 1:, :])
            ones0 = singles.tile([128, 8, 1], f16)
            nc.vector.memset(ones0, 1.0)
            sigwarm = singles.tile([1, 1], f16)
            nc.vector.memset(sigwarm, 0.0)
            nc.scalar.activation(sigwarm, sigwarm, SIG, bias=0.0, scale=1.0)
            m_sb = singles.tile([128, N_CHUNKS, 16], f16)
            nc.sync.dma_start(out=m_sb, in_=mt_d[:, :, :])
            # remaining slabs arrive via the (otherwise idle) GPSIMD DMA
            # queue so they do not queue behind the SP constants
            for i in range(1, N_SLABS):
                sl = bass.ts(i, SLAB)
                nc.gpsimd.dma_start(out=xt2[:, sl], in_=xt2_d[:, sl])

            state = {}

            def emit_zacts(sl, c0, c1, split=False):
                """z matmuls + one sigmoid for flat gate cols [c0, c1)."""
                gt = state[("gt", sl)]
                with tc.high_priority():
                    z = zpsum.tile([128, c1 - c0], f32, tag="z",
                                   name=f"z{sl}_{c0}")
                    for j in range(c0 // 512, c1 // 512):
                        ch, h = divmod(j, 2)
                        nc.tensor.matmul(
                            z[:, j * 512 - c0: j * 512 - c0 + 512],
                            lhsT=w_sb[:, ch, :],
                            rhs=xt2[:, sl * SLAB + h * 512:
                                    sl * SLAB + (h + 1) * 512],
                            start=True,
                            stop=True,
                        )
                    if split:
                        mid = (c0 + c1) // 2
                        nc.scalar.activation(gt[:, c0:mid], z[:, 0:mid - c0],
                                             SIG, bias=0.0, scale=1.0)
                        nc.scalar.activation(gt[:, mid:c1],
                                             z[:, mid - c0:c1 - c0],
                                             SIG, bias=0.0, scale=1.0)
                    else:
                        nc.scalar.activation(gt[:, c0:c1], z, SIG,
                                             bias=0.0, scale=1.0)

            def emit_shallow(sl):
                """gt chunk0 -> batch-major, cascade levels 0-6 -> p7t."""
                gt = state[("gt", sl)]
                gsh = work.tile([128, 8, 128], f16, tag="gsh", name=f"gsh{sl}")
                for g in range(8):
                    nc.sync.dma_start(
                        out=gsh[:, g, :],
                        in_=gt[:, g * 128: (g + 1) * 128],
                        transpose=True,
                    )
                prev = ones0[:, 0:8, :]
                for d in range(7):
                    n = 1 << d
                    cur = work.tile([128, 8, 2 * n], f16, tag=f"pb{d + 1}",
                                    name=f"pb{d + 1}_{sl}")
                    gl = gsh[:, :, n - 1: 2 * n - 1]
                    nc.vector.tensor_mul(cur[:, :, 0:n], prev, gl)
                    nc.vector.tensor_sub(cur[:, :, n: 2 * n], prev,
                                         cur[:, :, 0:n])
                    prev = cur
                p7t = work.tile([128, SLAB], f16, tag="p7t", name=f"p7t{sl}")
                for g in range(8):
                    nc.sync.dma_start(
                        out=p7t[:, g * 128: (g + 1) * 128],
                        in_=prev[:, g, :],
                        transpose=True,
                    )
                state[("p7t", sl)] = p7t

            def prefetch(nxt):
                state[("gt", nxt)] = gtpool.tile([128, GFLAT], f16, tag="gt",
                                                 name=f"gt{nxt}")
                emit_zacts(nxt, 0, SLAB)
                emit_shallow(nxt)

            def emit_slab(sl, last):
                gt = state[("gt", sl)]
                op = opsum.tile([128, 8, 16], f32, tag="op", name=f"op{sl}")
                osb = work.tile([128, 8, 16], f32, tag="osb", name=f"osb{sl}")
                F = {}

                def gch(ch, h0=0, h1=SLAB):
                    return gt[:, ch * SLAB + h0: ch * SLAB + h1]

                def ftile(name):
                    t = fwork.tile([128, SLAB], f16, tag=name,
                                   name=f"{name}_{sl}")
                    F[name] = t
                    return t

                def folds(name, m_idx, start=False, stop=False, bsubs=None):
                    # start=True only on the very first matmul of this op
                    # tile: it marks the whole 2KB PSUM zero-region pending,
                    # so each chain's first touch initializes and later
                    # touches accumulate. A second start would clobber
                    # sibling chains sharing the bank.
                    src = F[name]
                    for i, b in enumerate(range(8) if bsubs is None else bsubs):
                        nc.tensor.matmul(
                            op[:, b, 0:N_CLASS],
                            lhsT=src[:, b * 128: (b + 1) * 128],
                            rhs=m_sb[:, m_idx, 0:N_CLASS],
                            start=start and i == 0,
                            stop=stop,
                            skip_group_check=True,
                        )

                # sigmoid tiles for chunks 1-7: 4x1536 + 1x1024 flat columns
                bounds = [1024, 2560, 4096, 5632, 7168, 8192]
                for u in range(5):
                    emit_zacts(sl, bounds[u], bounds[u + 1],
                               split=(last and u == 4))
                    if u == 0:
                        # prefetch chunk-0 gates + shallow chains two windows
                        # ahead: the gsh/cascade/p7t chain has ~5us of
                        # latency (two DMA-semaphore hops + serial cascade)
                        if sl + 2 < N_SLABS:
                            prefetch(sl + 2)
                        # ch1 complete
                        p7t = state[("p7t", sl)]
                        l8 = ftile("l8")
                        nc.vector.tensor_mul(l8, p7t, gch(1))
                        r8 = ftile("r8")
                        nc.vector.tensor_sub(r8, p7t, l8)
                    elif u == 1:
                        # ch2 + ch3 complete
                        l9a = ftile("l9a")
                        nc.vector.tensor_mul(l9a, F["l8"], gch(2))
                        r9a = ftile("r9a")
                        nc.vector.tensor_sub(r9a, F["l8"], l9a)
                        l9b = ftile("l9b")
                        nc.gpsimd.tensor_mul(l9b, F["r8"], gch(3))
                        # r9b on DVE: it feeds q3, the tail-critical chain
                        r9b = ftile("r9b")
                        nc.vector.tensor_sub(r9b, F["r8"], l9b)
                        folds("l9a", 0, start=True)
                        folds("r9a", 1)
                        folds("l9b", 2)
                        folds("r9b", 3)
                    elif u == 2:
                        # ch4 complete
                        q0 = ftile("q0")
                        nc.gpsimd.tensor_mul(q0, F["l9a"], gch(4))
                        folds("q0", 4)
                    elif u == 3:
                        # ch5 + ch6 complete
                        q1 = ftile("q1")
                        nc.gpsimd.tensor_mul(q1, F["l9b"], gch(5))
                        q2 = ftile("q2")
                        nc.vector.tensor_mul(q2, F["r9a"], gch(6))
                        folds("q1", 5)
                        folds("q2", 6)
                    elif u == 4:
                        # ch7 complete
                        q3 = ftile("q3")
                        if last:
                            for h in range(2):
                                hs = bass.ts(h, 512)
                                nc.vector.tensor_mul(
                                    q3[:, hs], F["r9b"][:, hs],
                                    gch(7, h * 512, (h + 1) * 512))
                                folds("q3", 7, stop=True,
                                      bsubs=range(4 * h, 4 * h + 4))
                                bs = slice(4 * h, 4 * h + 4)
                                nc.vector.tensor_copy(
                                    osb[:, bs, :], op[:, bs, :])
                                # SP queue: the Pool engine can still be busy
                                # with a q-product at drain time; SP is idle
                                nc.sync.dma_start(
                                    out=out_d[:, sl, bs, :],
                                    in_=osb[:, bs, :],
                                )
                        else:
                            nc.vector.tensor_mul(q3, F["r9b"], gch(7))
                            folds("q3", 7, stop=True)
                            nc.vector.tensor_copy(osb, op)
                            nc.gpsimd.dma_start(
                                out=out_d[:, sl, :, :], in_=osb[:, :, :])

            # slab 0 chunk-0 as two 512-col piece pairs: the first sigmoid
            # fires as soon as the first half of x lands, and the second
            # half's z fill hides under the first sigmoid
            state[("gt", 0)] = gtpool.tile([128, GFLAT], f16,
                                           tag="gt", name="gt0")
            gt0 = state[("gt", 0)]
            with tc.high_priority():
                for h in range(2):
                    hs = bass.ts(h, 512)
                    zh = zpsum.tile([128, 512], f32, tag="z",
                                    name=f"z0_head{h}")
                    nc.tensor.matmul(zh, lhsT=w_sb[:, 0, :], rhs=xt2[:, hs],
                                     start=True, stop=True)
                    nc.scalar.activation(gt0[:, hs], zh, SIG,
                                         bias=0.0, scale=1.0)
            emit_shallow(0)
            state[("gt", 1)] = gtpool.tile([128, GFLAT], f16,
                                           tag="gt", name="gt1")
            emit_zacts(1, 0, SLAB)
            emit_shallow(1)
            for sl in range(N_SLABS):
                emit_slab(sl, last=(sl == N_SLABS - 1))

    nc.finalize()
    return nc


def _get_program():
    if "nc" not in _CACHE:
        _CACHE["nc"] = _build_program()
    return _CACHE["nc"]


def kernel(x, c, s, leaf_logits, dims, max_depth):
    from concourse.bass_utils import run_bass_kernel_spmd

    assert int(max_depth) == MAX_DEPTH
    x = np.asarray(x, dtype=F32)
    c = np.asarray(c, dtype=F32)
    s = np.asarray(s, dtype=F32)
    leaf_logits = np.asarray(leaf_logits, dtype=F32)
    dims = np.asarray(dims)

    W, lo_feats, M = _build_constants(c, s, dims, leaf_logits)
    wt = np.ascontiguousarray(W.transpose(1, 0, 2))            # [128, 8, 128]
    mt = np.ascontiguousarray(M.transpose(1, 0, 2))            # [128, 8, 16]

    in_maps = []
    for core in range(N_CORES):
        xc = x[core * B_CORE: (core + 1) * B_CORE]             # [8192, 64]
        xT = np.ascontiguousarray(xc.T).astype(F32)            # [64, 8192]
        x_hi = xT.astype(F16)
        x_lo = (xT - x_hi.astype(F32)).astype(F16)
        xt2 = np.empty((128, B_CORE), dtype=F16)               # [128, 8192]
        xt2[:IN_DIM] = x_hi
        xt2[IN_DIM:126] = x_lo[lo_feats]
        xt2[126:] = 1.0                                        # bias rows
        in_maps.append({"xt2": xt2, "wt": wt, "mt": mt})

    _CACHE["in_maps"] = in_maps
    nc = _get_program()
    res = run_bass_kernel_spmd(nc, in_maps, core_ids=list(range(N_CORES)))

    out = np.empty((B_FULL, N_CLASS), dtype=F32)
    for core in range(N_CORES):
        outt = res.results[core]["outt"]                # [128, 8, 8, 16] f32
        arr = outt[:, :, :, :N_CLASS].transpose(1, 2, 0, 3).reshape(
            B_CORE, N_CLASS)
        out[core * B_CORE: (core + 1) * B_CORE] = arr.astype(F32)
    return out
